# revision 4
# baseline (speedup 1.0000x reference)
"""Trainium2 Bass kernel for nn_Greedy_Base_hGLM.

Data-parallel over time T across 8 NeuronCores, no collectives: the
causal-conv halo is handled by overlapping input shards (256 extra
timesteps per core).

Per core:
  host:  REBAR reparam math on tiny [20,2500] params -> 3 C_syn variants;
         double-exp conv kernels -> block-Toeplitz lhsT matrices;
         shard S_e/S_i over T with halo, transpose to feature-major bf16
         (matmul contracts the partition dim; DMA-transpose is
         2-byte-only, so the layout is prepared host-side while sharding).
  device: projection matmuls (C_all stationary, S^T moving) -> in_e/in_i
         for all 3 variants at once; PE-transpose back to time-major;
         201-tap causal depthwise conv as block-Toeplitz matmuls;
         per-timestep tree-tanh recurrence (DVE/ACT); V for 3 variants.
  host:  reassemble [3, 20000]; small outputs (theta, hard_z, soft_z,
         soft_zb) from the host math.

Overlap structure: S^T is streamed as (j, chunk) pieces issued
chunk-major, so chunk 0's projection matmuls start ~8us in instead of
waiting for the whole 14 MB; conv+tree run in two block-halves, half A
overlapping the tail of the S stream.
"""

import numpy as np
import ml_dtypes

SUB_NO = 20
T_NO = 201
E_NO = 2000
I_NO = 500
T_DATA = 20000

N_CORES = 8
T_PER = 2500          # output timesteps owned per core
BLK = 128             # time block (= partition dim)
OUT_BLKS = 20         # ceil(2500/128) output blocks per core
HALO_BLKS = 2         # 256-step halo covers the 201-tap causal kernel
IN_BLKS = OUT_BLKS + HALO_BLKS          # 22
T_IN = IN_BLKS * BLK                    # 2816 input timesteps per core
HALO = HALO_BLKS * BLK                  # 256

E_PAD = 2048
I_PAD = 512
J_ROWS = E_PAD + I_PAD                  # 2560
J_TILES = J_ROWS // 128                 # 20
N_CH = 120            # (e/i) * 20 subunits * 3 variants
N_V = 3               # hard, soft, soft_b

# conv/tree processed in two block-halves for overlap with the S stream
HALF_BLKS = OUT_BLKS // 2               # 10 output blocks per half

BF16 = ml_dtypes.bfloat16


def _host_small_math(u, v, C_syn_log):
    """REBAR reparam: theta, hard_z, soft_z, soft_zb (all float32)."""
    x = C_syn_log - C_syn_log.max(axis=0, keepdims=True)
    ex = np.exp(x)
    theta = (ex / ex.sum(axis=0, keepdims=True)).astype(np.float32)
    rebar_z = np.log(theta) - np.log(-np.log(u))
    idx = np.argmax(rebar_z, axis=0)
    hard_z = np.zeros_like(rebar_z)
    hard_z[idx, np.arange(rebar_z.shape[1])] = 1.0
    v_k = np.sum(v * hard_z, axis=0, keepdims=True)
    z_same = -np.log(-np.log(v))
    z_diff = -np.log(-np.log(v) / theta - np.log(v_k))
    rebar_zb = hard_z * z_same + (1.0 - hard_z) * z_diff
    sig = lambda t: (1.0 / (1.0 + np.exp(-t / np.float32(0.5)))).astype(np.float32)
    soft_z = sig(rebar_z) + np.float32(1e-9)
    soft_zb = sig(rebar_zb) + np.float32(1e-9)
    return theta, hard_z.astype(np.float32), soft_z, soft_zb


def _conv_kernels(W_syn, Tau_syn, Delta_syn):
    """Double-exponential synaptic kernels kern[s, ei, tau]  (float32)."""
    t_raw = np.arange(T_NO, dtype=np.float32)
    t = np.maximum(t_raw[None, None, :] - np.exp(Delta_syn)[:, :, None], 0.0)
    t_tau = t / np.exp(Tau_syn)[:, :, None]
    return (t_tau * np.exp(-t_tau) * W_syn[:, :, None]).astype(np.float32)


def _toeplitz_lhsT(kern):
    """Causal conv as block matmuls over 128-step blocks:

      y[i+128b] = sum_d sum_j Td[i,j] x[j+128(b-d)],  Td[i,j]=kern[i-j+128d]

    Returns lhsT (= Td transposed, [j, i]) per live d; d's whose block is
    negligible (kernel support shorter than 128d) are dropped.
    """
    i = np.arange(BLK)[None, :]
    j = np.arange(BLK)[:, None]
    mats = {}
    gmax = np.abs(kern).max() + 1e-30
    d_list = []
    for d in range(3):
        tau = i - j + BLK * d                     # [j, i]
        mask = (tau >= 0) & (tau < T_NO)
        tauc = np.clip(tau, 0, T_NO - 1)
        md = kern[:, :, tauc] * mask[None, None]  # [s, ei, j, i]
        if np.abs(md).max() > 1e-7 * gmax:
            d_list.append(d)
            mats[d] = md.astype(np.float32)
    return mats, d_list


def _build_program(nd, d_list, w_sub, vo):
    """Trace + compile the SPMD Bass program (one NEFF, all 8 cores).

    w_sub / vo are python floats baked into the instruction stream (same
    values for every core, so still SPMD-safe).
    """
    import concourse.bacc as bacc
    import concourse.mybir as mybir
    from concourse import bass, tile

    f32 = mybir.dt.float32
    bf16 = mybir.dt.bfloat16
    Act = mybir.ActivationFunctionType
    Alu = mybir.AluOpType

    nc = bacc.Bacc("TRN2", target_bir_lowering=False, debug=False,
                   num_devices=N_CORES)

    sT_d = nc.dram_tensor("sT", [J_ROWS, T_IN], bf16, kind="ExternalInput")
    c_d = nc.dram_tensor("c_all", [128, J_TILES * N_CH], bf16,
                         kind="ExternalInput")
    t_d = nc.dram_tensor("toep", [128, nd * 2 * SUB_NO * 128], bf16,
                         kind="ExternalInput")
    id_d = nc.dram_tensor("ident", [N_CH, N_CH], f32, kind="ExternalInput")
    v_d = nc.dram_tensor("v_out", [BLK, OUT_BLKS * N_V], f32,
                         kind="ExternalOutput")

    CHUNKS = []
    off = 0
    while off < T_IN:
        w = min(512, T_IN - off)
        CHUNKS.append((off, w))
        off += w
    NCHUNK = len(CHUNKS)                       # 6 (5x512 + 256)

    # X slot ranges consumed by each conv half (slot = block index + 2)
    #   half 0: out blocks 0..9   -> slots 0..11   (chunks 0..2)
    #   half 1: out blocks 10..19 -> slots 8..21   (chunks 2..5)
    XA_SLOTS = 12
    XB_SLOTS = 14
    XB_BASE = 8

    with tile.TileContext(nc) as tc:
        with (
            tc.tile_pool(name="const", bufs=1) as cpool,
            tc.tile_pool(name="sres", bufs=1) as spool,
            tc.tile_pool(name="pei", bufs=1) as peipool,
            tc.tile_pool(name="xall", bufs=1) as xpool,
            tc.tile_pool(name="sub", bufs=1) as subpool,
            tc.tile_pool(name="tmp", bufs=3) as tmppool,
            tc.tile_pool(name="pproj", bufs=2, space=bass.MemorySpace.PSUM) as ppsum,
            tc.tile_pool(name="ptrans", bufs=2, space=bass.MemorySpace.PSUM) as tpsum,
            tc.tile_pool(name="pconv", bufs=4, space=bass.MemorySpace.PSUM) as kpsum,
        ):
            ident = cpool.tile([N_CH, N_CH], f32, tag="id")
            nc.sync.dma_start(ident[:], id_d[:])
            c_sb = cpool.tile([128, J_TILES, N_CH], bf16, tag="c")
            nc.sync.dma_start(c_sb[:], c_d[:].rearrange("p (j c) -> p j c",
                                                        j=J_TILES))

            # S^T pieces: one tile per (j, chunk), DMAs issued chunk-major
            # so early chunks finish early under fair-share DMA queues.
            s_pc = [[None] * NCHUNK for _ in range(J_TILES)]
            for ci, (off, w) in enumerate(CHUNKS):
                for j in range(J_TILES):
                    st = spool.tile([128, w], bf16, tag=f"s{j}_{ci}")
                    nc.sync.dma_start(
                        st[:], sT_d[j * 128:(j + 1) * 128, off:off + w])
                    s_pc[j][ci] = st
                if ci == 2:
                    # Toeplitz matrices needed by conv half A (~25us in);
                    # issued here so they don't delay chunks 0-2.
                    t_sb = cpool.tile([128, nd * 2 * SUB_NO, 128], bf16,
                                      tag="t")
                    nc.sync.dma_start(
                        t_sb[:],
                        t_d[:].rearrange("p (m i) -> p m i",
                                         m=nd * 2 * SUB_NO))

            # conv inputs, time-major bf16: [t_in_block, slot, channel]
            xA = xpool.tile([BLK, XA_SLOTS, N_CH], bf16, tag="xa")
            xB = xpool.tile([BLK, XB_SLOTS, N_CH], bf16, tag="xb")

            def conv_tree_half(h):
                """Block-Toeplitz conv + tree recurrence for one half."""
                xt = xA if h == 0 else xB
                sub = [None] * SUB_NO
                for s in range(SUB_NO - 1, -1, -1):
                    cp = kpsum.tile([BLK, HALF_BLKS, N_V], f32, tag="conv")
                    n_mm = 2 * nd
                    k = 0
                    for ei in range(2):
                        ch0 = ei * 60 + s * N_V
                        for di in range(nd):
                            d = d_list[di]
                            m = (ei * nd + di) * SUB_NO + s
                            if h == 0:
                                sl = 2 - d
                            else:
                                sl = 12 - d - XB_BASE
                            nc.tensor.matmul(
                                cp[:], t_sb[:, m, :],
                                xt[:, sl:sl + HALF_BLKS, ch0:ch0 + N_V],
                                start=(k == 0), stop=(k == n_mm - 1))
                            k += 1
                    so = subpool.tile([BLK, HALF_BLKS * N_V], f32,
                                      tag=f"sub{s}h{h}")
                    acc = cp[:].rearrange("p b v -> p (b v)")
                    kids = [c for c in (2 * s + 1, 2 * s + 2) if c < SUB_NO]
                    for c in kids:
                        tt = tmppool.tile([BLK, HALF_BLKS * N_V], f32,
                                          tag="tmp")
                        nc.vector.scalar_tensor_tensor(
                            tt[:], sub[c][:], float(w_sub[c]), acc,
                            op0=Alu.mult, op1=Alu.add)
                        acc = tt[:]
                    nc.scalar.activation(so[:], acc, Act.Tanh)
                    sub[s] = so
                nc.vector.tensor_scalar(
                    vt[:, h * HALF_BLKS * N_V:(h + 1) * HALF_BLKS * N_V],
                    sub[0][:], float(w_sub[0]), float(vo),
                    Alu.mult, Alu.add)

            vt = tmppool.tile([BLK, OUT_BLKS * N_V], f32, tag="vout")

            # ---- projection (contract j) + PE-transpose, chunk by chunk
            for ci, (off, w) in enumerate(CHUNKS):
                ps = ppsum.tile([N_CH, 512], f32, tag="proj")
                for j in range(J_TILES):
                    nc.tensor.matmul(
                        ps[:, :w], c_sb[:, j, :], s_pc[j][ci][:],
                        start=(j == 0), stop=(j == J_TILES - 1))
                pc = peipool.tile([N_CH, 512], f32, tag=f"pei{ci}")
                nc.vector.tensor_copy(pc[:, :w], ps[:, :w])
                for bb in range(w // BLK):
                    slot = off // BLK + bb
                    tp = tpsum.tile([BLK, N_CH], f32, tag="tp")
                    nc.tensor.transpose(
                        tp[:], pc[:, bb * BLK:(bb + 1) * BLK], ident[:])
                    if slot < XA_SLOTS:
                        nc.vector.tensor_copy(xA[:, slot, :], tp[:])
                    if slot >= XB_BASE:
                        nc.vector.tensor_copy(xB[:, slot - XB_BASE, :],
                                              tp[:])
                if ci == 2:
                    conv_tree_half(0)   # overlaps the chunk 3-5 S stream
            conv_tree_half(1)

            # vt cols: half0 -> blocks 0..9, half1 -> blocks 10..19,
            # both b-major/v-minor, i.e. already col = b*3+v overall.
            nc.sync.dma_start(v_d[:], vt[:])

    nc.compile()
    return nc


def kernel(S_e, S_i, u, v, W_syn, Tau_syn, Delta_syn, W_sub, V_o, Theta,
           C_syn_log):
    theta, hard_z, soft_z, soft_zb = _host_small_math(
        np.asarray(u, np.float32), np.asarray(v, np.float32),
        np.asarray(C_syn_log, np.float32))

    kern = _conv_kernels(np.asarray(W_syn, np.float32),
                         np.asarray(Tau_syn, np.float32),
                         np.asarray(Delta_syn, np.float32))
    mats, d_list = _toeplitz_lhsT(kern)
    nd = len(d_list)

    # ---- projection weights: [j, ch], ch = ei*60 + s*3 + v
    variants = (hard_z, soft_z, soft_zb)
    C_all = np.zeros((J_ROWS, N_CH), np.float32)
    for vi, cz in enumerate(variants):
        C_all[:E_NO, 0 * 60 + np.arange(SUB_NO) * N_V + vi] = cz[:, :E_NO].T
        C_all[E_PAD:E_PAD + I_NO, 60 + np.arange(SUB_NO) * N_V + vi] = \
            cz[:, E_NO:].T
    c_dev = np.ascontiguousarray(
        C_all.reshape(J_TILES, 128, N_CH).transpose(1, 0, 2)
    ).astype(BF16).reshape(128, J_TILES * N_CH)

    # ---- Toeplitz lhsT upload: [j, m, i], m = (ei*nd + di)*20 + s
    t_dev = np.zeros((128, nd * 2 * SUB_NO, 128), np.float32)
    for di, d in enumerate(d_list):
        for ei in range(2):
            for s in range(SUB_NO):
                t_dev[:, (ei * nd + di) * SUB_NO + s, :] = mats[d][s, ei]
    t_dev = t_dev.astype(BF16).reshape(128, -1)

    ident = np.eye(N_CH, dtype=np.float32)

    # ---- shard S over T (halo + tail padding via one zero-padded transpose)
    width = HALO + (N_CORES - 1) * T_PER + T_IN
    SeT = np.zeros((E_PAD, width), BF16)
    SeT[:E_NO, HALO:HALO + T_DATA] = np.asarray(S_e).astype(BF16).T
    SiT = np.zeros((I_PAD, width), BF16)
    SiT[:I_NO, HALO:HALO + T_DATA] = np.asarray(S_i).astype(BF16).T

    in_maps = []
    for c in range(N_CORES):
        lo = c * T_PER
        sT = np.empty((J_ROWS, T_IN), BF16)
        sT[:E_PAD] = SeT[:, lo:lo + T_IN]
        sT[E_PAD:] = SiT[:, lo:lo + T_IN]
        in_maps.append({
            "sT": sT, "c_all": c_dev, "toep": t_dev, "ident": ident,
        })

    nc = _build_program(nd, d_list, np.asarray(W_sub, np.float64),
                        float(np.asarray(V_o).reshape(-1)[0]))

    from concourse.bass_utils import run_bass_kernel_spmd
    res = run_bass_kernel_spmd(nc, in_maps, list(range(N_CORES)))

    V = np.empty((N_V, T_DATA), np.float32)
    for c in range(N_CORES):
        arr = np.asarray(res.results[c]["v_out"], np.float32)   # [128, 60]
        flat = arr.reshape(BLK, OUT_BLKS, N_V).transpose(1, 0, 2) \
                  .reshape(OUT_BLKS * BLK, N_V)
        V[:, c * T_PER:(c + 1) * T_PER] = flat[:T_PER].T

    return (V[0], V[1], V[2], theta, hard_z, soft_z, soft_zb)


# revision 8
# speedup vs baseline: 1.1273x; 1.1273x over previous
"""Trainium2 Bass kernel for nn_Greedy_Base_hGLM.

Data-parallel over time T across 8 NeuronCores, no collectives: the
causal-conv halo is handled by overlapping input shards (256 extra
timesteps per core).

Per core:
  host:  REBAR reparam math on tiny [20,2500] params -> 3 C_syn variants;
         double-exp conv kernels -> block-Toeplitz lhsT matrices;
         shard S_e/S_i over T with halo, transpose to feature-major bf16
         (matmul contracts the partition dim; DMA-transpose is
         2-byte-only, so the layout is prepared host-side while sharding).
  device: projection matmuls (C_all stationary, S^T moving) -> in_e/in_i
         for all 3 variants at once; PE-transpose back to time-major;
         201-tap causal depthwise conv as block-Toeplitz matmuls;
         per-timestep tree-tanh recurrence (DVE/ACT); V for 3 variants.
  host:  reassemble [3, 20000]; small outputs (theta, hard_z, soft_z,
         soft_zb) from the host math.

Overlap structure: S^T is streamed as (j, chunk) pieces issued
chunk-major, so chunk 0's projection matmuls start ~8us in instead of
waiting for the whole 14 MB; conv+tree run in two block-halves, half A
overlapping the tail of the S stream.
"""

import numpy as np
import ml_dtypes

SUB_NO = 20
T_NO = 201
E_NO = 2000
I_NO = 500
T_DATA = 20000

N_CORES = 8
T_PER = 2500          # output timesteps owned per core
BLK = 128             # time block (= partition dim)
OUT_BLKS = 20         # ceil(2500/128) output blocks per core
HALO_BLKS = 2         # 256-step halo covers the 201-tap causal kernel
IN_BLKS = OUT_BLKS + HALO_BLKS          # 22
T_IN = IN_BLKS * BLK                    # 2816 input timesteps per core
HALO = HALO_BLKS * BLK                  # 256

E_PAD = 2048
I_PAD = 512
J_ROWS = E_PAD + I_PAD                  # 2560
J_TILES = J_ROWS // 128                 # 20
N_CH = 120            # (e/i) * 20 subunits * 3 variants
N_V = 3               # hard, soft, soft_b

# conv/tree processed in two block-halves for overlap with the S stream
HALF_BLKS = OUT_BLKS // 2               # 10 output blocks per half

BF16 = ml_dtypes.bfloat16


def _host_small_math(u, v, C_syn_log):
    """REBAR reparam: theta, hard_z, soft_z, soft_zb (all float32)."""
    x = C_syn_log - C_syn_log.max(axis=0, keepdims=True)
    ex = np.exp(x)
    theta = (ex / ex.sum(axis=0, keepdims=True)).astype(np.float32)
    rebar_z = np.log(theta) - np.log(-np.log(u))
    idx = np.argmax(rebar_z, axis=0)
    hard_z = np.zeros_like(rebar_z)
    hard_z[idx, np.arange(rebar_z.shape[1])] = 1.0
    v_k = np.sum(v * hard_z, axis=0, keepdims=True)
    z_same = -np.log(-np.log(v))
    z_diff = -np.log(-np.log(v) / theta - np.log(v_k))
    rebar_zb = hard_z * z_same + (1.0 - hard_z) * z_diff
    sig = lambda t: (1.0 / (1.0 + np.exp(-t / np.float32(0.5)))).astype(np.float32)
    soft_z = sig(rebar_z) + np.float32(1e-9)
    soft_zb = sig(rebar_zb) + np.float32(1e-9)
    return theta, hard_z.astype(np.float32), soft_z, soft_zb


def _conv_kernels(W_syn, Tau_syn, Delta_syn):
    """Double-exponential synaptic kernels kern[s, ei, tau]  (float32)."""
    t_raw = np.arange(T_NO, dtype=np.float32)
    t = np.maximum(t_raw[None, None, :] - np.exp(Delta_syn)[:, :, None], 0.0)
    t_tau = t / np.exp(Tau_syn)[:, :, None]
    return (t_tau * np.exp(-t_tau) * W_syn[:, :, None]).astype(np.float32)


def _toeplitz_lhsT(kern):
    """Causal conv as block matmuls over 128-step blocks:

      y[i+128b] = sum_d sum_j Td[i,j] x[j+128(b-d)],  Td[i,j]=kern[i-j+128d]

    Returns lhsT (= Td transposed, [j, i]) per live d; d's whose block is
    negligible (kernel support shorter than 128d) are dropped.
    """
    i = np.arange(BLK)[None, :]
    j = np.arange(BLK)[:, None]
    mats = {}
    gmax = np.abs(kern).max() + 1e-30
    d_list = []
    for d in range(3):
        tau = i - j + BLK * d                     # [j, i]
        mask = (tau >= 0) & (tau < T_NO)
        tauc = np.clip(tau, 0, T_NO - 1)
        md = kern[:, :, tauc] * mask[None, None]  # [s, ei, j, i]
        if np.abs(md).max() > 1e-7 * gmax:
            d_list.append(d)
            mats[d] = md.astype(np.float32)
    return mats, d_list


def _build_program(nd, d_list, w_sub, vo):
    """Trace + compile the SPMD Bass program (one NEFF, all 8 cores).

    w_sub / vo are python floats baked into the instruction stream (same
    values for every core, so still SPMD-safe).
    """
    import concourse.bacc as bacc
    import concourse.mybir as mybir
    from concourse import bass, tile

    f32 = mybir.dt.float32
    bf16 = mybir.dt.bfloat16
    Act = mybir.ActivationFunctionType
    Alu = mybir.AluOpType

    nc = bacc.Bacc("TRN2", target_bir_lowering=False, debug=False,
                   num_devices=N_CORES)

    # S^T pre-packed host-side as (chunk, j) pieces, each a contiguous
    # [128, 512] block -> single-descriptor DMAs (a strided 512-col slice
    # costs 128 descriptor rows per issue and saturates the issuing
    # engine; that stretched DMA active 46->84us in the v2 trace).
    sT_d = nc.dram_tensor("sT", [6 * J_TILES * 128, 512], bf16,
                          kind="ExternalInput")
    c_d = nc.dram_tensor("c_all", [128, J_TILES * N_CH], bf16,
                         kind="ExternalInput")
    t_d = nc.dram_tensor("toep", [128, nd * 2 * SUB_NO * 128], bf16,
                         kind="ExternalInput")
    id_d = nc.dram_tensor("ident", [N_CH, N_CH], f32, kind="ExternalInput")
    v_d = nc.dram_tensor("v_out", [BLK, OUT_BLKS * N_V], f32,
                         kind="ExternalOutput")

    CHUNKS = []
    off = 0
    while off < T_IN:
        w = min(512, T_IN - off)
        CHUNKS.append((off, w))
        off += w
    NCHUNK = len(CHUNKS)                       # 6 (5x512 + 256)

    # X slot ranges consumed by each conv half (slot = block index + 2)
    #   half 0: out blocks 0..9   -> slots 0..11   (chunks 0..2)
    #   half 1: out blocks 10..19 -> slots 8..21   (chunks 2..5)
    XA_SLOTS = 12
    XB_SLOTS = 14
    XB_BASE = 8

    with tile.TileContext(nc) as tc:
        with (
            tc.tile_pool(name="const", bufs=1) as cpool,
            tc.tile_pool(name="sres", bufs=1) as spool,
            tc.tile_pool(name="pei", bufs=1) as peipool,
            tc.tile_pool(name="xall", bufs=1) as xpool,
            tc.tile_pool(name="sub", bufs=1) as subpool,
            tc.tile_pool(name="tmp", bufs=3) as tmppool,
            tc.tile_pool(name="pproj", bufs=2, space=bass.MemorySpace.PSUM) as ppsum,
            tc.tile_pool(name="ptrans", bufs=2, space=bass.MemorySpace.PSUM) as tpsum,
            tc.tile_pool(name="pconv", bufs=4, space=bass.MemorySpace.PSUM) as kpsum,
        ):
            ident = cpool.tile([N_CH, N_CH], f32, tag="id")
            nc.sync.dma_start(ident[:], id_d[:])
            c_sb = cpool.tile([128, J_TILES, N_CH], bf16, tag="c")
            nc.sync.dma_start(c_sb[:], c_d[:].rearrange("p (j c) -> p j c",
                                                        j=J_TILES))

            # S^T pieces: one tile per (j, chunk), DMAs issued chunk-major
            # so early chunks finish early under fair-share DMA queues.
            # Issues alternate between the two HWDGE engines (SP/ACT) to
            # halve per-engine descriptor-generation load.
            s_pc = [[None] * NCHUNK for _ in range(J_TILES)]
            for ci, (off, w) in enumerate(CHUNKS):
                for j in range(J_TILES):
                    st = spool.tile([128, 512], bf16, tag=f"s{j}_{ci}")
                    row0 = (ci * J_TILES + j) * 128
                    eng = nc.sync if (j % 2 == 0) else nc.scalar
                    eng.dma_start(st[:], sT_d[row0:row0 + 128, :])
                    s_pc[j][ci] = st
                if ci == 2:
                    # Toeplitz matrices needed by conv half A (~25us in);
                    # issued here so they don't delay chunks 0-2.
                    t_sb = cpool.tile([128, nd * 2 * SUB_NO, 128], bf16,
                                      tag="t")
                    nc.sync.dma_start(
                        t_sb[:],
                        t_d[:].rearrange("p (m i) -> p m i",
                                         m=nd * 2 * SUB_NO))

            # conv inputs, time-major bf16: [t_in_block, slot, channel]
            xA = xpool.tile([BLK, XA_SLOTS, N_CH], bf16, tag="xa")
            xB = xpool.tile([BLK, XB_SLOTS, N_CH], bf16, tag="xb")

            def conv_tree_half(h):
                """Block-Toeplitz conv + tree recurrence for one half."""
                xt = xA if h == 0 else xB
                sub = [None] * SUB_NO
                for s in range(SUB_NO - 1, -1, -1):
                    cp = kpsum.tile([BLK, HALF_BLKS, N_V], f32, tag="conv")
                    n_mm = 2 * nd
                    k = 0
                    for ei in range(2):
                        ch0 = ei * 60 + s * N_V
                        for di in range(nd):
                            d = d_list[di]
                            m = (ei * nd + di) * SUB_NO + s
                            if h == 0:
                                sl = 2 - d
                            else:
                                sl = 12 - d - XB_BASE
                            nc.tensor.matmul(
                                cp[:], t_sb[:, m, :],
                                xt[:, sl:sl + HALF_BLKS, ch0:ch0 + N_V],
                                start=(k == 0), stop=(k == n_mm - 1))
                            k += 1
                    so = subpool.tile([BLK, HALF_BLKS * N_V], f32,
                                      tag=f"sub{s}h{h}")
                    acc = cp[:].rearrange("p b v -> p (b v)")
                    kids = [c for c in (2 * s + 1, 2 * s + 2) if c < SUB_NO]
                    for c in kids:
                        tt = tmppool.tile([BLK, HALF_BLKS * N_V], f32,
                                          tag="tmp")
                        nc.vector.scalar_tensor_tensor(
                            tt[:], sub[c][:], float(w_sub[c]), acc,
                            op0=Alu.mult, op1=Alu.add)
                        acc = tt[:]
                    nc.scalar.activation(so[:], acc, Act.Tanh)
                    sub[s] = so
                nc.vector.tensor_scalar(
                    vt[:, h * HALF_BLKS * N_V:(h + 1) * HALF_BLKS * N_V],
                    sub[0][:], float(w_sub[0]), float(vo),
                    Alu.mult, Alu.add)

            vt = tmppool.tile([BLK, OUT_BLKS * N_V], f32, tag="vout")

            # ---- projection (contract j) + PE-transpose, chunk by chunk
            for ci, (off, w) in enumerate(CHUNKS):
                ps = ppsum.tile([N_CH, 512], f32, tag="proj")
                for j in range(J_TILES):
                    nc.tensor.matmul(
                        ps[:, :w], c_sb[:, j, :], s_pc[j][ci][:, :w],
                        start=(j == 0), stop=(j == J_TILES - 1))
                pc = peipool.tile([N_CH, 512], f32, tag=f"pei{ci}")
                nc.vector.tensor_copy(pc[:, :w], ps[:, :w])
                for bb in range(w // BLK):
                    slot = off // BLK + bb
                    tp = tpsum.tile([BLK, N_CH], f32, tag="tp")
                    nc.tensor.transpose(
                        tp[:], pc[:, bb * BLK:(bb + 1) * BLK], ident[:])
                    if slot < XA_SLOTS:
                        nc.vector.tensor_copy(xA[:, slot, :], tp[:])
                    if slot >= XB_BASE:
                        nc.vector.tensor_copy(xB[:, slot - XB_BASE, :],
                                              tp[:])
                if ci == 2:
                    conv_tree_half(0)   # overlaps the chunk 3-5 S stream
            conv_tree_half(1)

            # vt cols: half0 -> blocks 0..9, half1 -> blocks 10..19,
            # both b-major/v-minor, i.e. already col = b*3+v overall.
            nc.sync.dma_start(v_d[:], vt[:])

    nc.compile()
    return nc


def kernel(S_e, S_i, u, v, W_syn, Tau_syn, Delta_syn, W_sub, V_o, Theta,
           C_syn_log):
    theta, hard_z, soft_z, soft_zb = _host_small_math(
        np.asarray(u, np.float32), np.asarray(v, np.float32),
        np.asarray(C_syn_log, np.float32))

    kern = _conv_kernels(np.asarray(W_syn, np.float32),
                         np.asarray(Tau_syn, np.float32),
                         np.asarray(Delta_syn, np.float32))
    mats, d_list = _toeplitz_lhsT(kern)
    nd = len(d_list)

    # ---- projection weights: [j, ch], ch = ei*60 + s*3 + v
    variants = (hard_z, soft_z, soft_zb)
    C_all = np.zeros((J_ROWS, N_CH), np.float32)
    for vi, cz in enumerate(variants):
        C_all[:E_NO, 0 * 60 + np.arange(SUB_NO) * N_V + vi] = cz[:, :E_NO].T
        C_all[E_PAD:E_PAD + I_NO, 60 + np.arange(SUB_NO) * N_V + vi] = \
            cz[:, E_NO:].T
    c_dev = np.ascontiguousarray(
        C_all.reshape(J_TILES, 128, N_CH).transpose(1, 0, 2)
    ).astype(BF16).reshape(128, J_TILES * N_CH)

    # ---- Toeplitz lhsT upload: [j, m, i], m = (ei*nd + di)*20 + s
    t_dev = np.zeros((128, nd * 2 * SUB_NO, 128), np.float32)
    for di, d in enumerate(d_list):
        for ei in range(2):
            for s in range(SUB_NO):
                t_dev[:, (ei * nd + di) * SUB_NO + s, :] = mats[d][s, ei]
    t_dev = t_dev.astype(BF16).reshape(128, -1)

    ident = np.eye(N_CH, dtype=np.float32)

    # ---- shard S over T (halo + tail padding via one zero-padded transpose)
    width = HALO + (N_CORES - 1) * T_PER + T_IN
    SeT = np.zeros((E_PAD, width), BF16)
    SeT[:E_NO, HALO:HALO + T_DATA] = np.asarray(S_e).astype(BF16).T
    SiT = np.zeros((I_PAD, width), BF16)
    SiT[:I_NO, HALO:HALO + T_DATA] = np.asarray(S_i).astype(BF16).T

    in_maps = []
    for c in range(N_CORES):
        lo = c * T_PER
        sT = np.empty((J_ROWS, T_IN), BF16)
        sT[:E_PAD] = SeT[:, lo:lo + T_IN]
        sT[E_PAD:] = SiT[:, lo:lo + T_IN]
        # pack as contiguous (chunk, j) pieces [128, 512]; the last
        # (256-wide) chunk is zero-padded to 512
        pieces = np.zeros((6, J_TILES, 128, 512), BF16)
        s3 = sT.reshape(J_TILES, 128, T_IN)
        for ci in range(6):
            off = ci * 512
            w = min(512, T_IN - off)
            pieces[ci, :, :, :w] = s3[:, :, off:off + w]
        in_maps.append({
            "sT": pieces.reshape(6 * J_TILES * 128, 512),
            "c_all": c_dev, "toep": t_dev, "ident": ident,
        })

    nc = _build_program(nd, d_list, np.asarray(W_sub, np.float64),
                        float(np.asarray(V_o).reshape(-1)[0]))

    from concourse.bass_utils import run_bass_kernel_spmd
    res = run_bass_kernel_spmd(nc, in_maps, list(range(N_CORES)))

    V = np.empty((N_V, T_DATA), np.float32)
    for c in range(N_CORES):
        arr = np.asarray(res.results[c]["v_out"], np.float32)   # [128, 60]
        flat = arr.reshape(BLK, OUT_BLKS, N_V).transpose(1, 0, 2) \
                  .reshape(OUT_BLKS * BLK, N_V)
        V[:, c * T_PER:(c + 1) * T_PER] = flat[:T_PER].T

    return (V[0], V[1], V[2], theta, hard_z, soft_z, soft_zb)


# revision 10
# speedup vs baseline: 1.1292x; 1.0017x over previous
"""Trainium2 Bass kernel for nn_Greedy_Base_hGLM.

Data-parallel over time T across 8 NeuronCores, no collectives: the
causal-conv halo is handled by overlapping input shards (256 extra
timesteps per core).

Per core:
  host:  REBAR reparam math on tiny [20,2500] params -> 3 C_syn variants;
         double-exp conv kernels -> block-Toeplitz lhsT matrices;
         shard S_e/S_i over T with halo, transpose to feature-major bf16
         (matmul contracts the partition dim; DMA-transpose is
         2-byte-only, so the layout is prepared host-side while sharding).
  device: projection matmuls (C_all stationary, S^T moving) -> in_e/in_i
         for all 3 variants at once; PE-transpose back to time-major;
         201-tap causal depthwise conv as block-Toeplitz matmuls;
         per-timestep tree-tanh recurrence (DVE/ACT); V for 3 variants.
  host:  reassemble [3, 20000]; small outputs (theta, hard_z, soft_z,
         soft_zb) from the host math.

Overlap structure: S^T is streamed as (j, chunk) pieces issued
chunk-major, so chunk 0's projection matmuls start ~8us in instead of
waiting for the whole 14 MB; conv+tree run in two block-halves, half A
overlapping the tail of the S stream.
"""

import numpy as np
import ml_dtypes

SUB_NO = 20
T_NO = 201
E_NO = 2000
I_NO = 500
T_DATA = 20000

N_CORES = 8
T_PER = 2500          # output timesteps owned per core
BLK = 128             # time block (= partition dim)
OUT_BLKS = 20         # ceil(2500/128) output blocks per core
HALO_BLKS = 2         # 256-step halo covers the 201-tap causal kernel
IN_BLKS = OUT_BLKS + HALO_BLKS          # 22
T_IN = IN_BLKS * BLK                    # 2816 input timesteps per core
HALO = HALO_BLKS * BLK                  # 256

E_PAD = 2048
I_PAD = 512
J_ROWS = E_PAD + I_PAD                  # 2560
J_TILES = J_ROWS // 128                 # 20
N_CH = 120            # (e/i) * 20 subunits * 3 variants
N_V = 3               # hard, soft, soft_b

# conv/tree processed in two block-halves for overlap with the S stream
HALF_BLKS = OUT_BLKS // 2               # 10 output blocks per half

BF16 = ml_dtypes.bfloat16


def _host_small_math(u, v, C_syn_log):
    """REBAR reparam: theta, hard_z, soft_z, soft_zb (all float32)."""
    x = C_syn_log - C_syn_log.max(axis=0, keepdims=True)
    ex = np.exp(x)
    theta = (ex / ex.sum(axis=0, keepdims=True)).astype(np.float32)
    rebar_z = np.log(theta) - np.log(-np.log(u))
    idx = np.argmax(rebar_z, axis=0)
    hard_z = np.zeros_like(rebar_z)
    hard_z[idx, np.arange(rebar_z.shape[1])] = 1.0
    v_k = np.sum(v * hard_z, axis=0, keepdims=True)
    z_same = -np.log(-np.log(v))
    z_diff = -np.log(-np.log(v) / theta - np.log(v_k))
    rebar_zb = hard_z * z_same + (1.0 - hard_z) * z_diff
    sig = lambda t: (1.0 / (1.0 + np.exp(-t / np.float32(0.5)))).astype(np.float32)
    soft_z = sig(rebar_z) + np.float32(1e-9)
    soft_zb = sig(rebar_zb) + np.float32(1e-9)
    return theta, hard_z.astype(np.float32), soft_z, soft_zb


def _conv_kernels(W_syn, Tau_syn, Delta_syn):
    """Double-exponential synaptic kernels kern[s, ei, tau]  (float32)."""
    t_raw = np.arange(T_NO, dtype=np.float32)
    t = np.maximum(t_raw[None, None, :] - np.exp(Delta_syn)[:, :, None], 0.0)
    t_tau = t / np.exp(Tau_syn)[:, :, None]
    return (t_tau * np.exp(-t_tau) * W_syn[:, :, None]).astype(np.float32)


def _toeplitz_lhsT(kern):
    """Causal conv as block matmuls over 128-step blocks:

      y[i+128b] = sum_d sum_j Td[i,j] x[j+128(b-d)],  Td[i,j]=kern[i-j+128d]

    Returns lhsT (= Td transposed, [j, i]) per live d; d's whose block is
    negligible (kernel support shorter than 128d) are dropped.
    """
    i = np.arange(BLK)[None, :]
    j = np.arange(BLK)[:, None]
    mats = {}
    gmax = np.abs(kern).max() + 1e-30
    d_list = []
    for d in range(3):
        tau = i - j + BLK * d                     # [j, i]
        mask = (tau >= 0) & (tau < T_NO)
        tauc = np.clip(tau, 0, T_NO - 1)
        md = kern[:, :, tauc] * mask[None, None]  # [s, ei, j, i]
        if np.abs(md).max() > 1e-7 * gmax:
            d_list.append(d)
            mats[d] = md.astype(np.float32)
    return mats, d_list


def _build_program(nd, d_list, w_sub, vo):
    """Trace + compile the SPMD Bass program (one NEFF, all 8 cores).

    w_sub / vo are python floats baked into the instruction stream (same
    values for every core, so still SPMD-safe).
    """
    import concourse.bacc as bacc
    import concourse.mybir as mybir
    from concourse import bass, tile

    f32 = mybir.dt.float32
    bf16 = mybir.dt.bfloat16
    Act = mybir.ActivationFunctionType
    Alu = mybir.AluOpType

    nc = bacc.Bacc("TRN2", target_bir_lowering=False, debug=False,
                   num_devices=N_CORES)

    # S^T pre-packed host-side as (third, j) pieces, each a contiguous
    # [128, 1024/768] block. Few-but-contiguous DMAs: per-issue cost on
    # the HWDGE engines is ~0.4-0.8us regardless of size, so 60 issues
    # split over three engines keeps the queues fed (v2/v3 lesson).
    sTa_d = nc.dram_tensor("sTa", [2 * J_TILES * 128, 1024], bf16,
                           kind="ExternalInput")
    sTb_d = nc.dram_tensor("sTb", [J_TILES * 128, 768], bf16,
                           kind="ExternalInput")
    c_d = nc.dram_tensor("c_all", [128, J_TILES * N_CH], bf16,
                         kind="ExternalInput")
    t_d = nc.dram_tensor("toep", [128, nd * 2 * SUB_NO * 128], bf16,
                         kind="ExternalInput")
    id_d = nc.dram_tensor("ident", [N_CH, N_CH], f32, kind="ExternalInput")
    v_d = nc.dram_tensor("v_out", [BLK, OUT_BLKS * N_V], f32,
                         kind="ExternalOutput")

    # (third, offset-in-third, width); thirds are 1024/1024/768 cols
    CHUNKS = [(0, 0, 512), (0, 512, 512), (1, 0, 512), (1, 512, 512),
              (2, 0, 512), (2, 512, 256)]
    NCHUNK = len(CHUNKS)

    # X slot ranges consumed by each conv half (slot = block index + 2)
    #   half 0: out blocks 0..9   -> slots 0..11   (chunks 0..2)
    #   half 1: out blocks 10..19 -> slots 8..21   (chunks 2..5)
    XA_SLOTS = 12
    XB_SLOTS = 14
    XB_BASE = 8

    with tile.TileContext(nc) as tc:
        with (
            tc.tile_pool(name="const", bufs=1) as cpool,
            tc.tile_pool(name="sres", bufs=1) as spool,
            tc.tile_pool(name="pei", bufs=1) as peipool,
            tc.tile_pool(name="xall", bufs=1) as xpool,
            tc.tile_pool(name="sub", bufs=1) as subpool,
            tc.tile_pool(name="tmp", bufs=3) as tmppool,
            tc.tile_pool(name="pproj", bufs=2, space=bass.MemorySpace.PSUM) as ppsum,
            tc.tile_pool(name="ptrans", bufs=2, space=bass.MemorySpace.PSUM) as tpsum,
            tc.tile_pool(name="pconv", bufs=4, space=bass.MemorySpace.PSUM) as kpsum,
        ):
            ident = cpool.tile([N_CH, N_CH], f32, tag="id")
            nc.sync.dma_start(ident[:], id_d[:])
            c_sb = cpool.tile([128, J_TILES, N_CH], bf16, tag="c")
            nc.sync.dma_start(c_sb[:], c_d[:].rearrange("p (j c) -> p j c",
                                                        j=J_TILES))

            # S^T pieces: one tile per (j, third), issued third-major so
            # early thirds complete early; issues rotate over SP/ACT
            # (HWDGE) and GpSimd (SWDGE, descriptor-gen on the 8 Q7
            # cores) to spread the per-issue cost.
            s_pc = [[None] * 3 for _ in range(J_TILES)]
            engs = None
            for t3 in range(3):
                for j in range(J_TILES):
                    wdt = 768 if t3 == 2 else 1024
                    st = spool.tile([128, wdt], bf16, tag=f"s{j}_{t3}")
                    if t3 == 2:
                        src_ap = sTb_d[j * 128:(j + 1) * 128, :]
                    else:
                        row0 = (t3 * J_TILES + j) * 128
                        src_ap = sTa_d[row0:row0 + 128, :]
                    eng = (nc.sync, nc.scalar, nc.gpsimd)[j % 3]
                    eng.dma_start(st[:], src_ap)
                    s_pc[j][t3] = st
                if t3 == 0:
                    # Toeplitz matrices: needed by conv half A (~33us);
                    # issued here so they don't delay the first third.
                    t_sb = cpool.tile([128, nd * 2 * SUB_NO, 128], bf16,
                                      tag="t")
                    nc.sync.dma_start(
                        t_sb[:],
                        t_d[:].rearrange("p (m i) -> p m i",
                                         m=nd * 2 * SUB_NO))

            # conv inputs, time-major bf16: [t_in_block, slot, channel]
            xA = xpool.tile([BLK, XA_SLOTS, N_CH], bf16, tag="xa")
            xB = xpool.tile([BLK, XB_SLOTS, N_CH], bf16, tag="xb")

            def conv_tree_half(h):
                """Block-Toeplitz conv + tree recurrence for one half."""
                xt = xA if h == 0 else xB
                sub = [None] * SUB_NO
                for s in range(SUB_NO - 1, -1, -1):
                    cp = kpsum.tile([BLK, HALF_BLKS, N_V], f32, tag="conv")
                    n_mm = 2 * nd
                    k = 0
                    for ei in range(2):
                        ch0 = ei * 60 + s * N_V
                        for di in range(nd):
                            d = d_list[di]
                            m = (ei * nd + di) * SUB_NO + s
                            if h == 0:
                                sl = 2 - d
                            else:
                                sl = 12 - d - XB_BASE
                            nc.tensor.matmul(
                                cp[:], t_sb[:, m, :],
                                xt[:, sl:sl + HALF_BLKS, ch0:ch0 + N_V],
                                start=(k == 0), stop=(k == n_mm - 1))
                            k += 1
                    so = subpool.tile([BLK, HALF_BLKS * N_V], f32,
                                      tag=f"sub{s}h{h}")
                    acc = cp[:].rearrange("p b v -> p (b v)")
                    kids = [c for c in (2 * s + 1, 2 * s + 2) if c < SUB_NO]
                    for c in kids:
                        tt = tmppool.tile([BLK, HALF_BLKS * N_V], f32,
                                          tag="tmp")
                        nc.vector.scalar_tensor_tensor(
                            tt[:], sub[c][:], float(w_sub[c]), acc,
                            op0=Alu.mult, op1=Alu.add)
                        acc = tt[:]
                    nc.scalar.activation(so[:], acc, Act.Tanh)
                    sub[s] = so
                nc.vector.tensor_scalar(
                    vt[:, h * HALF_BLKS * N_V:(h + 1) * HALF_BLKS * N_V],
                    sub[0][:], float(w_sub[0]), float(vo),
                    Alu.mult, Alu.add)

            vt = tmppool.tile([BLK, OUT_BLKS * N_V], f32, tag="vout")

            # ---- projection (contract j) + PE-transpose, chunk by chunk
            for ci, (t3, off, w) in enumerate(CHUNKS):
                goff = t3 * 1024 + off
                ps = ppsum.tile([N_CH, 512], f32, tag="proj")
                for j in range(J_TILES):
                    nc.tensor.matmul(
                        ps[:, :w], c_sb[:, j, :],
                        s_pc[j][t3][:, off:off + w],
                        start=(j == 0), stop=(j == J_TILES - 1))
                pc = peipool.tile([N_CH, 512], f32, tag=f"pei{ci}")
                nc.vector.tensor_copy(pc[:, :w], ps[:, :w])
                for bb in range(w // BLK):
                    slot = goff // BLK + bb
                    tp = tpsum.tile([BLK, N_CH], f32, tag="tp")
                    nc.tensor.transpose(
                        tp[:], pc[:, bb * BLK:(bb + 1) * BLK], ident[:])
                    if slot < XA_SLOTS:
                        nc.vector.tensor_copy(xA[:, slot, :], tp[:])
                    if slot >= XB_BASE:
                        nc.vector.tensor_copy(xB[:, slot - XB_BASE, :],
                                              tp[:])
                if ci == 2:
                    conv_tree_half(0)   # overlaps the chunk 3-5 S stream
            conv_tree_half(1)

            # vt cols: half0 -> blocks 0..9, half1 -> blocks 10..19,
            # both b-major/v-minor, i.e. already col = b*3+v overall.
            nc.sync.dma_start(v_d[:], vt[:])

    nc.compile()
    return nc


def kernel(S_e, S_i, u, v, W_syn, Tau_syn, Delta_syn, W_sub, V_o, Theta,
           C_syn_log):
    theta, hard_z, soft_z, soft_zb = _host_small_math(
        np.asarray(u, np.float32), np.asarray(v, np.float32),
        np.asarray(C_syn_log, np.float32))

    kern = _conv_kernels(np.asarray(W_syn, np.float32),
                         np.asarray(Tau_syn, np.float32),
                         np.asarray(Delta_syn, np.float32))
    mats, d_list = _toeplitz_lhsT(kern)
    nd = len(d_list)

    # ---- projection weights: [j, ch], ch = ei*60 + s*3 + v
    variants = (hard_z, soft_z, soft_zb)
    C_all = np.zeros((J_ROWS, N_CH), np.float32)
    for vi, cz in enumerate(variants):
        C_all[:E_NO, 0 * 60 + np.arange(SUB_NO) * N_V + vi] = cz[:, :E_NO].T
        C_all[E_PAD:E_PAD + I_NO, 60 + np.arange(SUB_NO) * N_V + vi] = \
            cz[:, E_NO:].T
    c_dev = np.ascontiguousarray(
        C_all.reshape(J_TILES, 128, N_CH).transpose(1, 0, 2)
    ).astype(BF16).reshape(128, J_TILES * N_CH)

    # ---- Toeplitz lhsT upload: [j, m, i], m = (ei*nd + di)*20 + s
    t_dev = np.zeros((128, nd * 2 * SUB_NO, 128), np.float32)
    for di, d in enumerate(d_list):
        for ei in range(2):
            for s in range(SUB_NO):
                t_dev[:, (ei * nd + di) * SUB_NO + s, :] = mats[d][s, ei]
    t_dev = t_dev.astype(BF16).reshape(128, -1)

    ident = np.eye(N_CH, dtype=np.float32)

    # ---- shard S over T (halo + tail padding via one zero-padded transpose)
    width = HALO + (N_CORES - 1) * T_PER + T_IN
    SeT = np.zeros((E_PAD, width), BF16)
    SeT[:E_NO, HALO:HALO + T_DATA] = np.asarray(S_e).astype(BF16).T
    SiT = np.zeros((I_PAD, width), BF16)
    SiT[:I_NO, HALO:HALO + T_DATA] = np.asarray(S_i).astype(BF16).T

    in_maps = []
    for c in range(N_CORES):
        lo = c * T_PER
        sT = np.empty((J_ROWS, T_IN), BF16)
        sT[:E_PAD] = SeT[:, lo:lo + T_IN]
        sT[E_PAD:] = SiT[:, lo:lo + T_IN]
        # pack as contiguous (third, j) pieces: [128,1024]x2 + [128,768]
        s3 = sT.reshape(J_TILES, 128, T_IN)
        pa = np.empty((2, J_TILES, 128, 1024), BF16)
        pa[0] = s3[:, :, 0:1024]
        pa[1] = s3[:, :, 1024:2048]
        pb = np.ascontiguousarray(s3[:, :, 2048:2816])
        in_maps.append({
            "sTa": pa.reshape(2 * J_TILES * 128, 1024),
            "sTb": pb.reshape(J_TILES * 128, 768),
            "c_all": c_dev, "toep": t_dev, "ident": ident,
        })

    nc = _build_program(nd, d_list, np.asarray(W_sub, np.float64),
                        float(np.asarray(V_o).reshape(-1)[0]))

    from concourse.bass_utils import run_bass_kernel_spmd
    res = run_bass_kernel_spmd(nc, in_maps, list(range(N_CORES)))

    V = np.empty((N_V, T_DATA), np.float32)
    for c in range(N_CORES):
        arr = np.asarray(res.results[c]["v_out"], np.float32)   # [128, 60]
        flat = arr.reshape(BLK, OUT_BLKS, N_V).transpose(1, 0, 2) \
                  .reshape(OUT_BLKS * BLK, N_V)
        V[:, c * T_PER:(c + 1) * T_PER] = flat[:T_PER].T

    return (V[0], V[1], V[2], theta, hard_z, soft_z, soft_zb)


# revision 11
# speedup vs baseline: 1.3190x; 1.1680x over previous
"""Trainium2 Bass kernel for nn_Greedy_Base_hGLM.

Data-parallel over time T across 8 NeuronCores, no collectives: the
causal-conv halo is handled by overlapping input shards (256 extra
timesteps per core).

Per core:
  host:  REBAR reparam math on tiny [20,2500] params -> 3 C_syn variants;
         double-exp conv kernels -> block-Toeplitz lhsT matrices;
         shard S_e/S_i over T with halo, transpose to feature-major bf16
         (matmul contracts the partition dim; DMA-transpose is
         2-byte-only, so the layout is prepared host-side while sharding).
  device: projection matmuls (C_all stationary, S^T moving) -> in_e/in_i
         for all 3 variants at once; PE-transpose back to time-major;
         201-tap causal depthwise conv as block-Toeplitz matmuls;
         per-timestep tree-tanh recurrence (DVE/ACT); V for 3 variants.
  host:  reassemble [3, 20000]; small outputs (theta, hard_z, soft_z,
         soft_zb) from the host math.

Overlap structure: S^T is streamed as (j, chunk) pieces issued
chunk-major, so chunk 0's projection matmuls start ~8us in instead of
waiting for the whole 14 MB; conv+tree run in two block-halves, half A
overlapping the tail of the S stream.
"""

import numpy as np
import ml_dtypes

SUB_NO = 20
T_NO = 201
E_NO = 2000
I_NO = 500
T_DATA = 20000

N_CORES = 8
T_PER = 2500          # output timesteps owned per core
BLK = 128             # time block (= partition dim)
OUT_BLKS = 20         # ceil(2500/128) output blocks per core
HALO_BLKS = 2         # 256-step halo covers the 201-tap causal kernel
IN_BLKS = OUT_BLKS + HALO_BLKS          # 22
T_IN = IN_BLKS * BLK                    # 2816 input timesteps per core
HALO = HALO_BLKS * BLK                  # 256

E_PAD = 2048
I_PAD = 512
J_ROWS = E_PAD + I_PAD                  # 2560
J_TILES = J_ROWS // 128                 # 20
N_CH = 120            # (e/i) * 20 subunits * 3 variants
N_V = 3               # hard, soft, soft_b

# conv/tree processed in two block-halves for overlap with the S stream
HALF_BLKS = OUT_BLKS // 2               # 10 output blocks per half

BF16 = ml_dtypes.bfloat16


def _host_small_math(u, v, C_syn_log):
    """REBAR reparam: theta, hard_z, soft_z, soft_zb (all float32)."""
    x = C_syn_log - C_syn_log.max(axis=0, keepdims=True)
    ex = np.exp(x)
    theta = (ex / ex.sum(axis=0, keepdims=True)).astype(np.float32)
    rebar_z = np.log(theta) - np.log(-np.log(u))
    idx = np.argmax(rebar_z, axis=0)
    hard_z = np.zeros_like(rebar_z)
    hard_z[idx, np.arange(rebar_z.shape[1])] = 1.0
    v_k = np.sum(v * hard_z, axis=0, keepdims=True)
    z_same = -np.log(-np.log(v))
    z_diff = -np.log(-np.log(v) / theta - np.log(v_k))
    rebar_zb = hard_z * z_same + (1.0 - hard_z) * z_diff
    sig = lambda t: (1.0 / (1.0 + np.exp(-t / np.float32(0.5)))).astype(np.float32)
    soft_z = sig(rebar_z) + np.float32(1e-9)
    soft_zb = sig(rebar_zb) + np.float32(1e-9)
    return theta, hard_z.astype(np.float32), soft_z, soft_zb


def _conv_kernels(W_syn, Tau_syn, Delta_syn):
    """Double-exponential synaptic kernels kern[s, ei, tau]  (float32)."""
    t_raw = np.arange(T_NO, dtype=np.float32)
    t = np.maximum(t_raw[None, None, :] - np.exp(Delta_syn)[:, :, None], 0.0)
    t_tau = t / np.exp(Tau_syn)[:, :, None]
    return (t_tau * np.exp(-t_tau) * W_syn[:, :, None]).astype(np.float32)


def _toeplitz_lhsT(kern):
    """Causal conv as block matmuls over 128-step blocks:

      y[i+128b] = sum_d sum_j Td[i,j] x[j+128(b-d)],  Td[i,j]=kern[i-j+128d]

    Returns lhsT (= Td transposed, [j, i]) per live d; d's whose block is
    negligible (kernel support shorter than 128d) are dropped.
    """
    i = np.arange(BLK)[None, :]
    j = np.arange(BLK)[:, None]
    mats = {}
    gmax = np.abs(kern).max() + 1e-30
    d_list = []
    for d in range(3):
        tau = i - j + BLK * d                     # [j, i]
        mask = (tau >= 0) & (tau < T_NO)
        tauc = np.clip(tau, 0, T_NO - 1)
        md = kern[:, :, tauc] * mask[None, None]  # [s, ei, j, i]
        if np.abs(md).max() > 1e-7 * gmax:
            d_list.append(d)
            mats[d] = md.astype(np.float32)
    return mats, d_list


def _build_program(nd, d_list, w_sub, vo):
    """Trace + compile the SPMD Bass program (one NEFF, all 8 cores).

    w_sub / vo are python floats baked into the instruction stream (same
    values for every core, so still SPMD-safe).
    """
    import concourse.bacc as bacc
    import concourse.mybir as mybir
    from concourse import bass, tile

    f32 = mybir.dt.float32
    bf16 = mybir.dt.bfloat16
    Act = mybir.ActivationFunctionType
    Alu = mybir.AluOpType

    nc = bacc.Bacc("TRN2", target_bir_lowering=False, debug=False,
                   num_devices=N_CORES)

    # S^T pre-packed host-side as (half, j) pieces, each a contiguous
    # block: [128,1536] (half 0 = conv-half-A's 12 slots) / [128,1280].
    # Few-but-contiguous DMAs: per-issue cost on the HWDGE engines is
    # ~0.4-0.8us regardless of size (SWDGE ~1.6us), so 40 issues over
    # SP+ACT keep the queues fed while projection MMs chase arrivals.
    sTa_d = nc.dram_tensor("sTa", [J_TILES * 128, 1536], bf16,
                           kind="ExternalInput")
    sTb_d = nc.dram_tensor("sTb", [J_TILES * 128, 1280], bf16,
                           kind="ExternalInput")
    c_d = nc.dram_tensor("c_all", [128, J_TILES * N_CH], bf16,
                         kind="ExternalInput")
    t_d = nc.dram_tensor("toep", [128, nd * 2 * SUB_NO * 128], bf16,
                         kind="ExternalInput")
    id_d = nc.dram_tensor("ident", [N_CH, N_CH], f32, kind="ExternalInput")
    v_d = nc.dram_tensor("v_out", [BLK, OUT_BLKS * N_V], f32,
                         kind="ExternalOutput")

    # (half, offset-in-half, width); halves are 1536/1280 cols
    CHUNKS = [(0, 0, 512), (0, 512, 512), (0, 1024, 512), (1, 0, 512),
              (1, 512, 512), (1, 1024, 256)]
    NCHUNK = len(CHUNKS)

    # X slot ranges consumed by each conv half (slot = block index + 2)
    #   half 0: out blocks 0..9   -> slots 0..11   (chunks 0..2)
    #   half 1: out blocks 10..19 -> slots 8..21   (chunks 2..5)
    XA_SLOTS = 12
    XB_SLOTS = 14
    XB_BASE = 8

    with tile.TileContext(nc) as tc:
        with (
            tc.tile_pool(name="const", bufs=1) as cpool,
            tc.tile_pool(name="sres", bufs=1) as spool,
            tc.tile_pool(name="pei", bufs=1) as peipool,
            tc.tile_pool(name="xall", bufs=1) as xpool,
            tc.tile_pool(name="sub", bufs=1) as subpool,
            tc.tile_pool(name="tmp", bufs=3) as tmppool,
            tc.tile_pool(name="pproj", bufs=2, space=bass.MemorySpace.PSUM) as ppsum,
            tc.tile_pool(name="ptrans", bufs=2, space=bass.MemorySpace.PSUM) as tpsum,
            tc.tile_pool(name="pconv", bufs=4, space=bass.MemorySpace.PSUM) as kpsum,
        ):
            ident = cpool.tile([N_CH, N_CH], f32, tag="id")
            nc.sync.dma_start(ident[:], id_d[:])
            c_sb = cpool.tile([128, J_TILES, N_CH], bf16, tag="c")
            nc.sync.dma_start(c_sb[:], c_d[:].rearrange("p (j c) -> p j c",
                                                        j=J_TILES))

            # S^T pieces: one tile per (j, half), issued half-major so
            # the first half completes early; alternate the two HWDGE
            # engines (SP/ACT) to spread the per-issue cost.
            s_pc = [[None] * 2 for _ in range(J_TILES)]
            for h in range(2):
                hd, wdt = (sTa_d, 1536) if h == 0 else (sTb_d, 1280)
                for j in range(J_TILES):
                    st = spool.tile([128, wdt], bf16, tag=f"s{j}_{h}")
                    eng = nc.sync if (j % 2 == 0) else nc.scalar
                    eng.dma_start(st[:], hd[j * 128:(j + 1) * 128, :])
                    s_pc[j][h] = st
                if h == 0:
                    # Toeplitz matrices: needed by conv half A (~30us);
                    # issued here so they don't delay the first half.
                    t_sb = cpool.tile([128, nd * 2 * SUB_NO, 128], bf16,
                                      tag="t")
                    nc.sync.dma_start(
                        t_sb[:],
                        t_d[:].rearrange("p (m i) -> p m i",
                                         m=nd * 2 * SUB_NO))

            # conv inputs, time-major bf16: [t_in_block, slot, channel]
            xA = xpool.tile([BLK, XA_SLOTS, N_CH], bf16, tag="xa")
            xB = xpool.tile([BLK, XB_SLOTS, N_CH], bf16, tag="xb")

            def conv_tree_half(h):
                """Block-Toeplitz conv + tree recurrence for one half."""
                xt = xA if h == 0 else xB
                sub = [None] * SUB_NO
                for s in range(SUB_NO - 1, -1, -1):
                    cp = kpsum.tile([BLK, HALF_BLKS, N_V], f32, tag="conv")
                    n_mm = 2 * nd
                    k = 0
                    for ei in range(2):
                        ch0 = ei * 60 + s * N_V
                        for di in range(nd):
                            d = d_list[di]
                            m = (ei * nd + di) * SUB_NO + s
                            if h == 0:
                                sl = 2 - d
                            else:
                                sl = 12 - d - XB_BASE
                            nc.tensor.matmul(
                                cp[:], t_sb[:, m, :],
                                xt[:, sl:sl + HALF_BLKS, ch0:ch0 + N_V],
                                start=(k == 0), stop=(k == n_mm - 1))
                            k += 1
                    so = subpool.tile([BLK, HALF_BLKS * N_V], f32,
                                      tag=f"sub{s}h{h}")
                    acc = cp[:].rearrange("p b v -> p (b v)")
                    kids = [c for c in (2 * s + 1, 2 * s + 2) if c < SUB_NO]
                    for c in kids:
                        tt = tmppool.tile([BLK, HALF_BLKS * N_V], f32,
                                          tag="tmp")
                        nc.vector.scalar_tensor_tensor(
                            tt[:], sub[c][:], float(w_sub[c]), acc,
                            op0=Alu.mult, op1=Alu.add)
                        acc = tt[:]
                    nc.scalar.activation(so[:], acc, Act.Tanh)
                    sub[s] = so
                nc.vector.tensor_scalar(
                    vt[:, h * HALF_BLKS * N_V:(h + 1) * HALF_BLKS * N_V],
                    sub[0][:], float(w_sub[0]), float(vo),
                    Alu.mult, Alu.add)

            vt = tmppool.tile([BLK, OUT_BLKS * N_V], f32, tag="vout")

            # ---- projection (contract j) + PE-transpose, chunk by chunk
            for ci, (h, off, w) in enumerate(CHUNKS):
                goff = h * 1536 + off
                ps = ppsum.tile([N_CH, 512], f32, tag="proj")
                for j in range(J_TILES):
                    nc.tensor.matmul(
                        ps[:, :w], c_sb[:, j, :],
                        s_pc[j][h][:, off:off + w],
                        start=(j == 0), stop=(j == J_TILES - 1))
                pc = peipool.tile([N_CH, 512], f32, tag=f"pei{ci}")
                nc.vector.tensor_copy(pc[:, :w], ps[:, :w])
                for bb in range(w // BLK):
                    slot = goff // BLK + bb
                    tp = tpsum.tile([BLK, N_CH], f32, tag="tp")
                    nc.tensor.transpose(
                        tp[:], pc[:, bb * BLK:(bb + 1) * BLK], ident[:])
                    if slot < XA_SLOTS:
                        nc.vector.tensor_copy(xA[:, slot, :], tp[:])
                    if slot >= XB_BASE:
                        nc.vector.tensor_copy(xB[:, slot - XB_BASE, :],
                                              tp[:])
                if ci == 2:
                    conv_tree_half(0)   # overlaps the chunk 3-5 S stream
            conv_tree_half(1)

            # vt cols: half0 -> blocks 0..9, half1 -> blocks 10..19,
            # both b-major/v-minor, i.e. already col = b*3+v overall.
            nc.sync.dma_start(v_d[:], vt[:])

    nc.compile()
    return nc


def kernel(S_e, S_i, u, v, W_syn, Tau_syn, Delta_syn, W_sub, V_o, Theta,
           C_syn_log):
    theta, hard_z, soft_z, soft_zb = _host_small_math(
        np.asarray(u, np.float32), np.asarray(v, np.float32),
        np.asarray(C_syn_log, np.float32))

    kern = _conv_kernels(np.asarray(W_syn, np.float32),
                         np.asarray(Tau_syn, np.float32),
                         np.asarray(Delta_syn, np.float32))
    mats, d_list = _toeplitz_lhsT(kern)
    nd = len(d_list)

    # ---- projection weights: [j, ch], ch = ei*60 + s*3 + v
    variants = (hard_z, soft_z, soft_zb)
    C_all = np.zeros((J_ROWS, N_CH), np.float32)
    for vi, cz in enumerate(variants):
        C_all[:E_NO, 0 * 60 + np.arange(SUB_NO) * N_V + vi] = cz[:, :E_NO].T
        C_all[E_PAD:E_PAD + I_NO, 60 + np.arange(SUB_NO) * N_V + vi] = \
            cz[:, E_NO:].T
    c_dev = np.ascontiguousarray(
        C_all.reshape(J_TILES, 128, N_CH).transpose(1, 0, 2)
    ).astype(BF16).reshape(128, J_TILES * N_CH)

    # ---- Toeplitz lhsT upload: [j, m, i], m = (ei*nd + di)*20 + s
    t_dev = np.zeros((128, nd * 2 * SUB_NO, 128), np.float32)
    for di, d in enumerate(d_list):
        for ei in range(2):
            for s in range(SUB_NO):
                t_dev[:, (ei * nd + di) * SUB_NO + s, :] = mats[d][s, ei]
    t_dev = t_dev.astype(BF16).reshape(128, -1)

    ident = np.eye(N_CH, dtype=np.float32)

    # ---- shard S over T (halo + tail padding via one zero-padded transpose)
    width = HALO + (N_CORES - 1) * T_PER + T_IN
    SeT = np.zeros((E_PAD, width), BF16)
    SeT[:E_NO, HALO:HALO + T_DATA] = np.asarray(S_e).astype(BF16).T
    SiT = np.zeros((I_PAD, width), BF16)
    SiT[:I_NO, HALO:HALO + T_DATA] = np.asarray(S_i).astype(BF16).T

    in_maps = []
    for c in range(N_CORES):
        lo = c * T_PER
        sT = np.empty((J_ROWS, T_IN), BF16)
        sT[:E_PAD] = SeT[:, lo:lo + T_IN]
        sT[E_PAD:] = SiT[:, lo:lo + T_IN]
        # pack as contiguous (half, j) pieces: [128,1536] + [128,1280]
        s3 = sT.reshape(J_TILES, 128, T_IN)
        pa = np.ascontiguousarray(s3[:, :, :1536])
        pb = np.ascontiguousarray(s3[:, :, 1536:])
        in_maps.append({
            "sTa": pa.reshape(J_TILES * 128, 1536),
            "sTb": pb.reshape(J_TILES * 128, 1280),
            "c_all": c_dev, "toep": t_dev, "ident": ident,
        })

    nc = _build_program(nd, d_list, np.asarray(W_sub, np.float64),
                        float(np.asarray(V_o).reshape(-1)[0]))

    from concourse.bass_utils import run_bass_kernel_spmd
    res = run_bass_kernel_spmd(nc, in_maps, list(range(N_CORES)))

    V = np.empty((N_V, T_DATA), np.float32)
    for c in range(N_CORES):
        arr = np.asarray(res.results[c]["v_out"], np.float32)   # [128, 60]
        flat = arr.reshape(BLK, OUT_BLKS, N_V).transpose(1, 0, 2) \
                  .reshape(OUT_BLKS * BLK, N_V)
        V[:, c * T_PER:(c + 1) * T_PER] = flat[:T_PER].T

    return (V[0], V[1], V[2], theta, hard_z, soft_z, soft_zb)
